# revision 12
# baseline (speedup 1.0000x reference)
"""MultiPropMLP (MoE-routed tiny MLP) Trainium2 kernel — host-routed version.

Problem: out[n] = MLP_{idx[n]}(xs[n]) for N = 8192*128 samples, K = 8 experts,
MLP = 16 -> 64 -> relu -> 64 -> relu -> 1 with per-expert weights.

Sharding strategy (the hint is advisory; we choose expert-major): the host
sorts samples by expert and deals them across the 8 cores, so each core's
Bass program is a fully static schedule of single-expert tiles — the device
never sees idxs and computes exactly one expert per sample (the staged
baseline computed all 8 and masked, wasting 8x engine time).

Packing: 2 samples per PE column via block-diag duplicated weights.
A column = 32 rows: rows 0:16 = lane-0 sample features, 16:32 = lane-1.
  layer0: lhsT = diag2(W0_k) [32,128], rhs = x columns    -> h0 [128, 512]
  layer1: lhsT = diag2(W1_k) [128,128], rhs = h0          -> h1 [128, 512]
  layer2: lhsT = W2 pair at cols 2s,2s+1 of [128,32]      -> accumulates into
          partition pair 2s of a shared [32,512] PSUM bank (s = g % 16)
All matmuls f32r (free dim 512 -> 1 cycle/row). Per 512-col group (1024
samples): 3 matmuls (PE ~645ns) + 2 relu-bias PSUM evacs (one ACT, one DVE).
This walrus rejects matmul PSUM dst at partition base != 0 (no col tiling),
so layer-2 outputs are instead *accumulated* 16 groups deep into one bank
via shifted stationaries (the 30 zero columns accumulate nothing), giving a
dense [32,512] evac every 16 groups (~40ns/group) instead of a sparse
[2,512] evac per group (~600ns). b2 is added on the host while unpacking.

Layout per core: samples sorted by expert, 2 lanes (even/odd per core
split), padded to CAP=8704 column-pairs per expert. Logical column
L = k*CAP + c; stored in x4 [128, 2*CAP]: expert k occupies partition rows
32*(k//2):+32, columns (k%2)*CAP + c. Groups run expert-major: g = 17k + j,
group columns 512j..512j+512 of expert k's segment.

Output: group g, lane r, column c -> out_c[2*(g%16) + r, 512*(g//16) + c];
host inverts the permutation and adds b2.

Note: walrus in this toolchain accepts only ONE sync-wait per instruction;
_split_ctrl_waits() hoists Tile's multi-waits onto single-wait nops.
"""

import numpy as np

R, S, D_IN, WIDTH, K = 8192, 128, 16, 64, 8
N = R * S
NCORES = 8
NC = N // NCORES                  # 131072 samples per core
CAP0 = 8704                       # default column-pairs per (core, expert)
BANKG = 16                        # groups accumulated per l2 PSUM bank

_cache = {}


def _build_nc(cap):
    import concourse.bass as bass
    import concourse.mybir as mybir
    from concourse import tile

    f32 = mybir.dt.float32
    f32r = mybir.dt.float32r
    Relu = mybir.ActivationFunctionType.Relu
    add = mybir.AluOpType.add
    mx = mybir.AluOpType.max

    qcols = 2 * cap               # columns per quadrant (2 experts)
    gpe = cap // 512              # groups per expert (17 for CAP0)
    ng = K * gpe                  # total groups (136)
    nbank = -(-ng // BANKG)       # l2 banks (9)

    # weight table layout (columns of wts / w_sb):
    #   w0: [0, 256)        expert k=2q+e at rows 32q:32q+32, cols 128e:128e+128
    #   w1: [256, 1280)     diag2(W1_k) at cols 256+128k
    #   w2: [1280, 5376)    block (k,s): cols 1280+32*(16k+s), pair at 2s,2s+1
    W0OFF, W1OFF, W2OFF, WCOLS = 0, 256, 1280, 5376

    nc = bass.Bass()
    x4_c = nc.dram_tensor("x4_c", [128, qcols], f32, kind="ExternalInput")
    wts = nc.dram_tensor("wts", [128, WCOLS], f32, kind="ExternalInput")
    bia = nc.dram_tensor("bia", [128, 16], f32, kind="ExternalInput")
    out_c = nc.dram_tensor("out_c", [32, 512 * nbank], f32, kind="ExternalOutput")

    with tile.TileContext(nc) as tc:
        with (
            tc.tile_pool(name="const", bufs=1) as cpool,
            tc.tile_pool(name="xs", bufs=1) as xpool,
            tc.tile_pool(name="work", bufs=4) as wpool,
            tc.tile_pool(name="ostage", bufs=1) as opool,
            tc.tile_pool(name="ps_h", bufs=3, space="PSUM") as ps_h,
            tc.tile_pool(name="ps_l2", bufs=2, space="PSUM") as ps_l2,
        ):
            w_sb = cpool.tile([128, WCOLS], f32r, tag="wts")
            b_sb = cpool.tile([128, 16], f32, tag="bias")
            x4 = xpool.tile([128, qcols], f32r, tag="x4")
            stage = opool.tile([32, 512 * nbank], f32, tag="ostage")

            # DMA order tuned for start latency: first expert's first columns
            # and the weights it needs come first; everything else streams
            # behind while compute runs. f32 -> f32r casts ride gpsimd SWDGE.
            nc.gpsimd.dma_start(w_sb[:, W0OFF : W0OFF + 256], wts[:, W0OFF : W0OFF + 256])
            nc.gpsimd.dma_start(x4[0:32, 0:1024], x4_c[0:32, 0:1024])
            nc.sync.dma_start(b_sb[:], bia[:])
            nc.gpsimd.dma_start(
                w_sb[:, W1OFF : W1OFF + 128], wts[:, W1OFF : W1OFF + 128]
            )
            nc.gpsimd.dma_start(
                w_sb[:, W2OFF : W2OFF + 512], wts[:, W2OFF : W2OFF + 512]
            )
            nc.gpsimd.dma_start(x4[0:32, 1024:cap], x4_c[0:32, 1024:cap])
            nc.gpsimd.dma_start(
                w_sb[:, W1OFF + 128 : W1OFF + 1024], wts[:, W1OFF + 128 : W1OFF + 1024]
            )
            for k in range(1, K):
                q, e = k // 2, k % 2
                nc.gpsimd.dma_start(
                    x4[32 * q : 32 * q + 32, e * cap : (e + 1) * cap],
                    x4_c[32 * q : 32 * q + 32, e * cap : (e + 1) * cap],
                )
                nc.gpsimd.dma_start(
                    w_sb[:, W2OFF + 512 * k : W2OFF + 512 * (k + 1)],
                    wts[:, W2OFF + 512 * k : W2OFF + 512 * (k + 1)],
                )

            # Software-pipelined emission: engines execute strictly in program
            # order, so at step t we issue h0mm(t), h1mm(t-1), l2mm(t-2) — each
            # instruction's cross-engine dependency then has a full group
            # period to land, and the PE never head-blocks on an evac.
            def kq(g):
                k = g // gpe
                return k, k // 2, k % 2, (k % 2) * cap + 512 * (g % gpe)

            h0ps_t, h0_t, h1ps_t, h1_t = {}, {}, {}, {}
            l2_ps = [None]

            def emit_h0mm(g):
                k, q, e, c0 = kq(g)
                h0ps_t[g] = ps_h.tile([128, 512], f32, tag="h0ps", name=f"h0ps{g}")
                nc.tensor.matmul(
                    h0ps_t[g][:],
                    w_sb[32 * q : 32 * q + 32, W0OFF + 128 * e : W0OFF + 128 * (e + 1)],
                    x4[32 * q : 32 * q + 32, c0 : c0 + 512],
                    start=True, stop=True, tile_position=(32 * q, 0),
                )

            def emit_h0evac(g):
                k, q, e, c0 = kq(g)
                h0_t[g] = wpool.tile([128, 512], f32r, tag="h0", name=f"h0_{g}")
                nc.scalar.activation(
                    h0_t[g][:], h0ps_t.pop(g)[:], Relu, bias=b_sb[:, k : k + 1]
                )

            def emit_h1mm(g):
                k, q, e, c0 = kq(g)
                h1ps_t[g] = ps_h.tile([128, 512], f32, tag="h1ps", name=f"h1ps{g}")
                nc.tensor.matmul(
                    h1ps_t[g][:],
                    w_sb[:, W1OFF + 128 * k : W1OFF + 128 * (k + 1)],
                    h0_t.pop(g)[:],
                    start=True, stop=True, tile_position=(0, 0),
                )

            def emit_h1evac(g):
                k, q, e, c0 = kq(g)
                h1_t[g] = wpool.tile([128, 512], f32r, tag="h1", name=f"h1_{g}")
                nc.vector.tensor_scalar(
                    h1_t[g][:], h1ps_t.pop(g)[:], b_sb[:, 8 + k : 9 + k], 0.0, add, mx
                )

            def emit_l2mm(g):
                k, q, e, c0 = kq(g)
                s = g % BANKG
                if s == 0:
                    l2_ps[0] = ps_l2.tile([32, 512], f32, tag="l2", name=f"l2_{g}")
                tcol = W2OFF + 32 * (BANKG * k + s)
                last = s == BANKG - 1 or g == ng - 1
                nc.tensor.matmul(
                    l2_ps[0][:],
                    w_sb[:, tcol : tcol + 32],
                    h1_t.pop(g)[:],
                    start=(s == 0), stop=last, tile_position=(0, 0),
                )
                if last:
                    b = g // BANKG
                    nc.scalar.copy(stage[:, 512 * b : 512 * (b + 1)], l2_ps[0][:])

            for t in range(ng + 2):
                if t < ng:
                    emit_h0mm(t)
                if 1 <= t < ng + 1:
                    emit_h1mm(t - 1)
                if 2 <= t:
                    emit_l2mm(t - 2)
                if t < ng:
                    emit_h0evac(t)
                if 1 <= t < ng + 1:
                    emit_h1evac(t - 1)

            nc.sync.dma_start(out_c[:], stage[:])

    _split_ctrl_waits(nc, mybir)
    return nc


def _split_ctrl_waits(nc, mybir):
    """walrus in this container accepts only one sync-wait per instruction;
    Tile attaches one wait per dependency lane. Hoist extras onto preceding
    single-wait nops on the same engine (equivalent ordering semantics)."""
    for bb in nc.main_func.blocks:
        newlist = []
        changed = False
        for ins in bb.instructions:
            si = ins.sync_info
            if si is not None and len(si.on_wait) > 1:
                waits = list(si.on_wait)
                for j, w in enumerate(waits[:-1]):
                    nop = mybir.InstNoOp(name=f"{ins.name}-wsplit-{j}", ins=[], outs=[])
                    nop.engine = ins.engine
                    nop.sync_info = mybir.SyncInfo(on_wait=[w], on_update=[])
                    newlist.append(nop)
                si.on_wait = [waits[-1]]
                ins.sync_info = si
                changed = True
            newlist.append(ins)
        if changed:
            bb.instructions = newlist
    return nc


def _prep_consts(W0, b0, W1, b1, W2, b2):
    f = np.float32
    wts = np.zeros((128, 5376), f)
    bia = np.zeros((128, 16), f)
    for k in range(K):
        q, e = k // 2, k % 2
        wts[32 * q : 32 * q + 16, 128 * e : 128 * e + 64] = W0[k]
        wts[32 * q + 16 : 32 * q + 32, 128 * e + 64 : 128 * e + 128] = W0[k]
        wts[0:64, 256 + 128 * k : 256 + 128 * k + 64] = W1[k]
        wts[64:128, 256 + 128 * k + 64 : 256 + 128 * k + 128] = W1[k]
        for s in range(BANKG):
            tcol = 1280 + 32 * (BANKG * k + s)
            wts[0:64, tcol + 2 * s] = W2[k, :, 0]
            wts[64:128, tcol + 2 * s + 1] = W2[k, :, 0]
        bia[0:64, k] = b0[k]
        bia[64:128, k] = b0[k]
        bia[0:64, 8 + k] = b1[k]
        bia[64:128, 8 + k] = b1[k]
    return wts, bia


def kernel(idxs, xs, W0, b0, W1, b1, W2, b2):
    from concourse.bass_utils import run_bass_kernel_spmd

    idx_flat = np.asarray(idxs).reshape(N)
    xs_flat = np.ascontiguousarray(np.asarray(xs, np.float32).reshape(N, D_IN))

    order = np.argsort(idx_flat, kind="stable")
    counts = np.bincount(idx_flat, minlength=K)

    # capacity (multiple of 512 column-pairs) that fits every (core, expert)
    max_part = -(-int(counts.max()) // NCORES)        # samples per (core, expert)
    need = -(-max_part // 2)                          # column-pairs
    cap = max(CAP0, -(-need // 512) * 512)
    if ("nc", cap) not in _cache:
        _cache[("nc", cap)] = _build_nc(cap)
    nc = _cache[("nc", cap)]
    _cache["nc"] = nc                                 # test.py reads this
    qcols = 2 * cap
    gpe = cap // 512
    ng = K * gpe
    nbank = -(-ng // BANKG)

    wts, bia = _prep_consts(
        np.asarray(W0), np.asarray(b0), np.asarray(W1), np.asarray(b1),
        np.asarray(W2), np.asarray(b2),
    )

    # slot[m, k, c, lane] = global sample index routed there (-1 = padding)
    slot = np.full((NCORES, K, cap, 2), -1, np.int64)
    pos = 0
    for k in range(K):
        ids_k = order[pos : pos + counts[k]]
        pos += counts[k]
        for m, p in enumerate(np.array_split(ids_k, NCORES)):
            top, bot = p[0::2], p[1::2]
            slot[m, k, : len(top), 0] = top
            slot[m, k, : len(bot), 1] = bot
    msk = slot >= 0
    xp = np.zeros((NCORES, K, cap, 2, D_IN), np.float32)
    xp[msk] = xs_flat[slot[msk]]

    in_maps = []
    for m in range(NCORES):
        # expert k -> rows 32*(k//2):+32, cols (k%2)*cap
        x4 = xp[m].reshape(4, qcols, 32).transpose(0, 2, 1).reshape(128, qcols)
        in_maps.append(dict(x4_c=np.ascontiguousarray(x4), wts=wts, bia=bia))

    res = run_bass_kernel_spmd(nc, in_maps, list(range(NCORES))).results

    b2v = np.asarray(b2, np.float32)[:, 0]
    out = np.empty(N, np.float32)
    for m in range(NCORES):
        oc = np.asarray(res[m]["out_c"])              # [32, 512*nbank]
        # group g (= k*gpe + j), lane r, col c -> oc[2*(g%16)+r, 512*(g//16)+c]
        banks = oc.reshape(BANKG, 2, nbank, 512)      # [s, r, b, c]
        vals = banks.transpose(2, 0, 3, 1).reshape(nbank * BANKG, 512, 2)
        vals = vals[:ng].reshape(K, cap, 2)           # [k, c_seg, lane]
        vals = vals + b2v[:, None, None]
        out[slot[m][msk[m]]] = vals[msk[m]]
    return out.reshape(R, S, 1)


# revision 25
# speedup vs baseline: 1.1259x; 1.1259x over previous
"""MultiPropMLP (MoE-routed tiny MLP) Trainium2 kernel — host-routed version.

Problem: out[n] = MLP_{idx[n]}(xs[n]) for N = 8192*128 samples, K = 8 experts,
MLP = 16 -> 64 -> relu -> 64 -> relu -> 1 with per-expert weights.

Sharding strategy (the hint is advisory; we choose expert-major): the host
sorts samples by expert and deals them across the 8 cores, so each core's
Bass program is a fully static schedule of single-expert tiles — the device
never sees idxs and computes exactly one expert per sample (the staged
baseline computed all 8 and masked, wasting 8x engine time).

Packing: 2 samples per PE column via block-diag duplicated weights.
A column = 32 rows: rows 0:16 = lane-0 sample features, 16:32 = lane-1.
  layer0: lhsT = diag2(W0_k) [32,128], rhs = x columns    -> h0 [128, 512]
  layer1: lhsT = diag2(W1_k) [128,128], rhs = h0          -> h1 [128, 512]
  layer2: lhsT = W2 pair at cols 2s,2s+1 of [128,32]      -> accumulates into
          partition pair 2s of a shared [32,512] PSUM bank (s = g % 16)
All matmuls f32r (free dim 512 -> 1 cycle/row). Per 512-col group (1024
samples): 3 matmuls (PE ~645ns) + 2 relu-bias PSUM evacs (one ACT, one DVE).
This walrus rejects matmul PSUM dst at partition base != 0 (no col tiling),
so layer-2 outputs are instead *accumulated* 16 groups deep into one bank
via shifted stationaries (the 30 zero columns accumulate nothing), giving a
dense [32,512] evac every 16 groups (~40ns/group) instead of a sparse
[2,512] evac per group (~600ns). b2 is added on the host while unpacking.

Layout per core: samples sorted by expert, 2 lanes (even/odd per core
split), padded to CAP=8704 column-pairs per expert. Logical column
L = k*CAP + c; stored in x4 [128, 2*CAP]: expert k occupies partition rows
32*(k//2):+32, columns (k%2)*CAP + c. Groups run expert-major: g = 17k + j,
group columns 512j..512j+512 of expert k's segment.

Output: group g, lane r, column c -> out_c[2*(g%16) + r, 512*(g//16) + c];
host inverts the permutation and adds b2.

Note: walrus in this toolchain accepts only ONE sync-wait per instruction;
_split_ctrl_waits() hoists Tile's multi-waits onto single-wait nops.
"""

import numpy as np

R, S, D_IN, WIDTH, K = 8192, 128, 16, 64, 8
N = R * S
NCORES = 8
NC = N // NCORES                  # 131072 samples per core
CAP0 = 8704                       # default column-pairs per (core, expert)
BANKG = 16                        # groups accumulated per l2 PSUM bank

_cache = {}
LABELS = {}  # instruction name -> semantic label (debug aid)


def _build_nc(cap, bankg=None, h0bufs=3, h1bufs=3, wbufs=4, stage_eng="act", chains=1, l2bufs=2, l2off=2, stage_delay=0):
    import concourse.bass as bass
    import concourse.mybir as mybir
    from concourse import tile

    f32 = mybir.dt.float32
    f32r = mybir.dt.float32r
    Relu = mybir.ActivationFunctionType.Relu
    add = mybir.AluOpType.add
    mx = mybir.AluOpType.max

    bankg = bankg or BANKG
    qcols = 2 * cap               # columns per quadrant (2 experts)
    gpe = cap // 512              # groups per expert (17 for CAP0)
    ng = K * gpe                  # total groups (136)
    nbank = -(-ng // (chains * bankg)) * chains   # l2 banks

    # weight table layout (columns of wts / w_sb):
    #   w0: [0, 256)        expert k=2q+e at rows 32q:32q+32, cols 128e:128e+128
    #   w1: [256, 1280)     diag2(W1_k) at cols 256+128k
    #   w2: [1280, ...)     block (k,s) width 2*bankg, pair at cols 2s,2s+1
    bw = 2 * bankg
    W0OFF, W1OFF, W2OFF = 0, 256, 1280
    WCOLS = W2OFF + K * bankg * bw

    nc = bass.Bass()
    x4_c = nc.dram_tensor("x4_c", [128, qcols], f32r, kind="ExternalInput")
    wts = nc.dram_tensor("wts", [128, WCOLS], f32r, kind="ExternalInput")
    bia = nc.dram_tensor("bia", [128, 16], f32, kind="ExternalInput")
    out_c = nc.dram_tensor("out_c", [2 * bankg, 512 * nbank], f32, kind="ExternalOutput")

    with tile.TileContext(nc) as tc:
        with (
            tc.tile_pool(name="const", bufs=1) as cpool,
            tc.tile_pool(name="xs", bufs=1) as xpool,
            tc.tile_pool(name="work", bufs=wbufs) as wpool,
            tc.tile_pool(name="ostage", bufs=1) as opool,
            tc.tile_pool(name="ps_h0", bufs=h0bufs, space="PSUM") as ps_h0,
            tc.tile_pool(name="ps_h1", bufs=h1bufs, space="PSUM") as ps_h1,
            tc.tile_pool(name="ps_l2", bufs=l2bufs, space="PSUM") as ps_l2,
        ):
            w_sb = cpool.tile([128, WCOLS], f32r, tag="wts")
            b_sb = cpool.tile([128, 16], f32, tag="bias")
            x4 = xpool.tile([128, qcols], f32r, tag="x4")
            stage = opool.tile([2 * bankg, 512 * nbank], f32, tag="ostage")

            # DMA order tuned for start latency: the first column blocks
            # (all four quadrants) and the even experts' weights come first;
            # everything else streams behind compute. DRAM tensors are f32r
            # (same bits as f32) so loads ride the sync/HWDGE path.
            kw = bankg * bw               # w2 table cols per expert
            def w1dma(k):
                nc.sync.dma_start(
                    w_sb[:, W1OFF + 128 * k : W1OFF + 128 * (k + 1)],
                    wts[:, W1OFF + 128 * k : W1OFF + 128 * (k + 1)],
                )
            def w2dma(k):
                nc.sync.dma_start(
                    w_sb[:, W2OFF + kw * k : W2OFF + kw * (k + 1)],
                    wts[:, W2OFF + kw * k : W2OFF + kw * (k + 1)],
                )
            def x4dma(c0, c1):
                nc.sync.dma_start(x4[:, c0:c1], x4_c[:, c0:c1])
            nc.sync.dma_start(w_sb[:, W0OFF : W0OFF + 256], wts[:, W0OFF : W0OFF + 256])
            x4dma(0, 512)
            nc.sync.dma_start(b_sb[:], bia[:])
            w1dma(0); w2dma(0); w1dma(2); w2dma(2)
            x4dma(512, 1024)
            w1dma(4); w2dma(4); w1dma(6); w2dma(6)
            x4dma(1024, 2048)
            for k in (1, 3, 5, 7):
                w1dma(k)
            for k in (1, 3, 5, 7):
                w2dma(k)
            for c0 in range(2048, qcols, 2048):
                x4dma(c0, min(c0 + 2048, qcols))

            # Software-pipelined emission: engines execute strictly in program
            # order, so at step t we issue h0mm(t), h1mm(t-1), l2mm(t-2) — each
            # instruction's cross-engine dependency then has a full group
            # period to land, and the PE never head-blocks on an evac.
            def kq(g):
                # quadrant-rotating order: q = g%4, column block cb = g//4
                q, cb = g % 4, g // 4
                k = 2 * q + (1 if cb >= gpe else 0)
                return k, q, k % 2, 512 * cb

            h0ps_t, h0_t, h1ps_t, h1_t = {}, {}, {}, {}
            l2_ps = [None] * 8
            pending_stage = []
            stage_due = {}

            def _mk(fn, lbl):
                def g(*a, **kw):
                    r = fn(*a, **kw)
                    try:
                        LABELS[r.ins.name] = lbl
                    except Exception:
                        pass
                    return r
                return g

            def _rec(lbl):
                return _mk(nc.tensor.matmul, lbl)

            def _reca(lbl):
                return _mk(nc.scalar.activation, lbl)

            def _recv(lbl):
                return _mk(nc.vector.tensor_scalar, lbl)

            def _recc(lbl):
                return _mk(nc.scalar.copy, lbl)

            def _reccv(lbl):
                return _mk(nc.vector.tensor_copy, lbl)

            def emit_h0mm(g):
                k, q, e, c0 = kq(g)
                h0ps_t[g] = ps_h0.tile([128, 512], f32, tag="h0ps", name=f"h0ps{g}")
                _rec(f"h0mm({g})")(
                    h0ps_t[g][:],
                    w_sb[32 * q : 32 * q + 32, W0OFF + 128 * e : W0OFF + 128 * (e + 1)],
                    x4[32 * q : 32 * q + 32, c0 : c0 + 512],
                    start=True, stop=True, tile_position=(32 * q, 0),
                )

            def emit_h0evac(g):
                k, q, e, c0 = kq(g)
                h0_t[g] = wpool.tile([128, 512], f32r, tag="h0", name=f"h0_{g}")
                _reca(f"h0evac({g})")(
                    h0_t[g][:], h0ps_t.pop(g)[:], Relu, bias=b_sb[:, k : k + 1]
                )

            def emit_h1mm(g):
                k, q, e, c0 = kq(g)
                h1ps_t[g] = ps_h1.tile([128, 512], f32, tag="h1ps", name=f"h1ps{g}")
                _rec(f"h1mm({g})")(
                    h1ps_t[g][:],
                    w_sb[:, W1OFF + 128 * k : W1OFF + 128 * (k + 1)],
                    h0_t.pop(g)[:],
                    start=True, stop=True, tile_position=(0, 0),
                )

            def emit_h1evac(g):
                k, q, e, c0 = kq(g)
                h1_t[g] = wpool.tile([128, 512], f32r, tag="h1", name=f"h1_{g}")
                _recv(f"h1evac({g})")(
                    h1_t[g][:], h1ps_t.pop(g)[:], b_sb[:, 8 + k : 9 + k], 0.0, add, mx
                )

            def emit_l2mm(g):
                k, q, e, c0 = kq(g)
                # chain c = g % chains; slot s = (g // chains) % bankg;
                # bank index b = g's window: groups fill `chains` banks per
                # chains*bankg window, then all are staged.
                cch = g % chains
                s = (g // chains) % bankg
                if s == 0 and cch == 0:
                    for i in range(chains):
                        l2_ps[i] = ps_l2.tile(
                            [2 * bankg, 512], f32, tag="l2", name=f"l2_{g}_{i}"
                        )
                tcol = W2OFF + bw * (bankg * k + s)
                last = s == bankg - 1 or g >= ng - chains
                _rec(f"l2mm({g})")(
                    l2_ps[cch][:],
                    w_sb[:, tcol : tcol + bw],
                    h1_t.pop(g)[:],
                    start=(s == 0), stop=last, tile_position=(0, 0),
                )
                if last:
                    b = (g // (chains * bankg)) * chains + cch
                    pending_stage.append((b, l2_ps[cch]))

            def emit_stage():
                b, tile_ = pending_stage.pop(0)
                cp = _recc if stage_eng == "act" else _reccv
                cp(f"stage({b})")(stage[:, 512 * b : 512 * (b + 1)], tile_[:])
                if (b + 1) % 3 == 0 or b == nbank - 1:
                    o0 = 512 * ((b // 3) * 3)
                    o1 = 512 * (b + 1)
                    nc.gpsimd.dma_start(out_c[:, o0:o1], stage[:, o0:o1])

            for t in range(ng + l2off + stage_delay + 1):
                if t < ng:
                    emit_h0mm(t)
                if 1 <= t < ng + 1:
                    emit_h1mm(t - 1)
                if l2off <= t < ng + l2off:
                    g2 = t - l2off
                    emit_l2mm(g2)
                    if pending_stage and g2 not in stage_due:
                        # bank finished at step t; stage it stage_delay later
                        stage_due[t + stage_delay] = True
                while pending_stage and (stage_due.pop(t, False) or t >= ng + l2off):
                    emit_stage()
                    break
                if t < ng:
                    emit_h0evac(t)
                if 1 <= t < ng + 1:
                    emit_h1evac(t - 1)

    _split_ctrl_waits(nc, mybir)
    return nc


def _split_ctrl_waits(nc, mybir):
    """walrus in this container accepts only one sync-wait per instruction;
    Tile attaches one wait per dependency lane. Hoist extras onto preceding
    single-wait nops on the same engine (equivalent ordering semantics)."""
    for bb in nc.main_func.blocks:
        newlist = []
        changed = False
        for ins in bb.instructions:
            si = ins.sync_info
            if si is not None and len(si.on_wait) > 1:
                waits = list(si.on_wait)
                for j, w in enumerate(waits[:-1]):
                    nop = mybir.InstNoOp(name=f"{ins.name}-wsplit-{j}", ins=[], outs=[])
                    nop.engine = ins.engine
                    nop.sync_info = mybir.SyncInfo(on_wait=[w], on_update=[])
                    newlist.append(nop)
                si.on_wait = [waits[-1]]
                ins.sync_info = si
                changed = True
            newlist.append(ins)
        if changed:
            bb.instructions = newlist
    return nc


def _prep_consts(W0, b0, W1, b1, W2, b2, bankg=None):
    f = np.float32
    bankg = bankg or BANKG
    bw = 2 * bankg
    wts = np.zeros((128, 1280 + K * bankg * bw), f)
    bia = np.zeros((128, 16), f)
    for k in range(K):
        q, e = k // 2, k % 2
        wts[32 * q : 32 * q + 16, 128 * e : 128 * e + 64] = W0[k]
        wts[32 * q + 16 : 32 * q + 32, 128 * e + 64 : 128 * e + 128] = W0[k]
        wts[0:64, 256 + 128 * k : 256 + 128 * k + 64] = W1[k]
        wts[64:128, 256 + 128 * k + 64 : 256 + 128 * k + 128] = W1[k]
        for s in range(bankg):
            tcol = 1280 + bw * (bankg * k + s)
            wts[0:64, tcol + 2 * s] = W2[k, :, 0]
            wts[64:128, tcol + 2 * s + 1] = W2[k, :, 0]
        bia[0:64, k] = b0[k]
        bia[64:128, k] = b0[k]
        bia[0:64, 8 + k] = b1[k]
        bia[64:128, 8 + k] = b1[k]
    return wts, bia


def kernel(idxs, xs, W0, b0, W1, b1, W2, b2):
    from concourse.bass_utils import run_bass_kernel_spmd

    idx_flat = np.asarray(idxs).reshape(N)
    xs_flat = np.ascontiguousarray(np.asarray(xs, np.float32).reshape(N, D_IN))

    order = np.argsort(idx_flat, kind="stable")
    counts = np.bincount(idx_flat, minlength=K)

    # capacity (multiple of 512 column-pairs) that fits every (core, expert)
    max_part = -(-int(counts.max()) // NCORES)        # samples per (core, expert)
    need = -(-max_part // 2)                          # column-pairs
    cap = max(CAP0, -(-need // 512) * 512)
    if ("nc", cap) not in _cache:
        _cache[("nc", cap)] = _build_nc(cap)
    nc = _cache[("nc", cap)]
    _cache["nc"] = nc                                 # test.py reads this
    qcols = 2 * cap
    gpe = cap // 512
    ng = K * gpe
    nbank = -(-ng // BANKG)

    wts, bia = _prep_consts(
        np.asarray(W0), np.asarray(b0), np.asarray(W1), np.asarray(b1),
        np.asarray(W2), np.asarray(b2),
    )

    # slot[m, k, c, lane] = global sample index routed there (-1 = padding)
    slot = np.full((NCORES, K, cap, 2), -1, np.int64)
    pos = 0
    for k in range(K):
        ids_k = order[pos : pos + counts[k]]
        pos += counts[k]
        for m, p in enumerate(np.array_split(ids_k, NCORES)):
            top, bot = p[0::2], p[1::2]
            slot[m, k, : len(top), 0] = top
            slot[m, k, : len(bot), 1] = bot
    msk = slot >= 0
    xp = np.zeros((NCORES, K, cap, 2, D_IN), np.float32)
    xp[msk] = xs_flat[slot[msk]]

    in_maps = []
    for m in range(NCORES):
        # expert k -> rows 32*(k//2):+32, cols (k%2)*cap
        x4 = xp[m].reshape(4, qcols, 32).transpose(0, 2, 1).reshape(128, qcols)
        in_maps.append(dict(x4_c=np.ascontiguousarray(x4), wts=wts, bia=bia))

    res = run_bass_kernel_spmd(nc, in_maps, list(range(NCORES))).results

    b2v = np.asarray(b2, np.float32)[:, 0]
    # group order is quadrant-rotating: group g covers quadrant q = g%4,
    # column block cb = g//4 (expert k = 2q + (cb>=gpe)); its lane-r value
    # for column c sits at oc[2*(g%BANKG)+r, 512*(g//BANKG)+c]
    out = np.empty(N, np.float32)
    gidx = np.arange(ng)
    q_g, cb_g = gidx % 4, gidx // 4
    k_g = 2 * q_g + (cb_g >= gpe)
    j_g = cb_g % gpe
    s_g, b_g = gidx % BANKG, gidx // BANKG
    for m in range(NCORES):
        oc = np.asarray(res[m]["out_c"])              # [2*BANKG, 512*nbank]
        vals = np.empty((K, cap, 2), np.float32)
        for g in range(ng):
            blk = oc[2 * s_g[g] : 2 * s_g[g] + 2, 512 * b_g[g] : 512 * b_g[g] + 512]
            vals[k_g[g], 512 * j_g[g] : 512 * (j_g[g] + 1)] = blk.T
        vals = vals + b2v[:, None, None]
        out[slot[m][msk[m]]] = vals[msk[m]]
    return out.reshape(R, S, 1)


# revision 35
# speedup vs baseline: 1.1822x; 1.0500x over previous
"""MultiPropMLP (MoE-routed tiny MLP) Trainium2 kernel — host-routed version.

Problem: out[n] = MLP_{idx[n]}(xs[n]) for N = 8192*128 samples, K = 8 experts,
MLP = 16 -> 64 -> relu -> 64 -> relu -> 1 with per-expert weights.

Sharding strategy (the hint is advisory; we choose expert-major): the host
sorts samples by expert and deals them across the 8 cores, so each core's
Bass program is a fully static schedule of single-expert tiles — the device
never sees idxs and computes exactly one expert per sample (the staged
baseline computed all 8 and masked, wasting 8x engine time).

Packing: 2 samples per PE column via block-diag duplicated weights.
A column = 32 rows: rows 0:16 = lane-0 sample features, 16:32 = lane-1.
  layer0: lhsT = diag2(W0_k) [32,128], rhs = x columns    -> h0 [128, 512]
  layer1: lhsT = diag2(W1_k) [128,128], rhs = h0          -> h1 [128, 512]
  layer2: lhsT = W2 pair at cols 2s,2s+1 of [128,32]      -> accumulates into
          partition pair 2s of a shared [32,512] PSUM bank (s = g % 16)
All matmuls f32r (free dim 512 -> 1 cycle/row). Per 512-col group (1024
samples): 3 matmuls (PE ~645ns) + 2 relu-bias PSUM evacs (one ACT, one DVE).
This walrus rejects matmul PSUM dst at partition base != 0 (no col tiling),
so layer-2 outputs are instead *accumulated* 16 groups deep into one bank
via shifted stationaries (the 30 zero columns accumulate nothing), giving a
dense [32,512] evac every 16 groups (~40ns/group) instead of a sparse
[2,512] evac per group (~600ns). b2 is added on the host while unpacking.

Layout per core: samples sorted by expert, 2 lanes (even/odd per core
split), padded to CAP=8704 column-pairs per expert. Logical column
L = k*CAP + c; stored in x4 [128, 2*CAP]: expert k occupies partition rows
32*(k//2):+32, columns (k%2)*CAP + c. Groups run expert-major: g = 17k + j,
group columns 512j..512j+512 of expert k's segment.

Output: group g, lane r, column c -> out_c[2*(g%16) + r, 512*(g//16) + c];
host inverts the permutation and adds b2.

Note: walrus in this toolchain accepts only ONE sync-wait per instruction;
_split_ctrl_waits() hoists Tile's multi-waits onto single-wait nops.
"""

import numpy as np

R, S, D_IN, WIDTH, K = 8192, 128, 16, 64, 8
N = R * S
NCORES = 8
NC = N // NCORES                  # 131072 samples per core
CAP0 = 8704                       # default column-pairs per (core, expert)
BANKG = 16                        # groups accumulated per l2 PSUM bank

_cache = {}
LABELS = {}  # instruction name -> semantic label (debug aid)


def _build_nc(cap, bankg=None, h0bufs=3, h1bufs=3, wbufs=4, stage_eng="act", chains=1, l2bufs=2, l2off=3, stage_delay=6, h1pair=False):
    import concourse.bass as bass
    import concourse.mybir as mybir
    from concourse import tile

    f32 = mybir.dt.float32
    f32r = mybir.dt.float32r
    Relu = mybir.ActivationFunctionType.Relu
    add = mybir.AluOpType.add
    mx = mybir.AluOpType.max

    bankg = bankg or BANKG
    qcols = 2 * cap               # columns per quadrant (2 experts)
    gpe = cap // 512              # groups per expert (17 for CAP0)
    ng = K * gpe                  # total groups (136)
    nbank = -(-ng // (chains * bankg)) * chains   # l2 banks

    # weight table layout (columns of wts / w_sb):
    #   w0: [0, 256)        expert k=2q+e at rows 32q:32q+32, cols 128e:128e+128
    #   w1: [256, 1280)     diag2(W1_k) at cols 256+128k
    #   w2: [1280, ...)     block (k,s) width 2*bankg, pair at cols 2s,2s+1
    bw = 2 * bankg
    W0OFF, W1OFF, W2OFF = 0, 256, 1280
    WCOLS = W2OFF + K * bankg * bw

    nc = bass.Bass()
    x4_c = nc.dram_tensor("x4_c", [128, qcols], f32r, kind="ExternalInput")
    wts = nc.dram_tensor("wts", [128, WCOLS], f32r, kind="ExternalInput")
    bia = nc.dram_tensor("bia", [128, 16], f32, kind="ExternalInput")
    out_c = nc.dram_tensor("out_c", [2 * bankg, 512 * nbank], f32, kind="ExternalOutput")

    with tile.TileContext(nc) as tc:
        with (
            tc.tile_pool(name="const", bufs=1) as cpool,
            tc.tile_pool(name="xs", bufs=1) as xpool,
            tc.tile_pool(name="work", bufs=wbufs) as wpool,
            tc.tile_pool(name="ostage", bufs=1) as opool,
            tc.tile_pool(name="ps_h0", bufs=h0bufs, space="PSUM") as ps_h0,
            tc.tile_pool(name="ps_h1", bufs=h1bufs, space="PSUM") as ps_h1,
            tc.tile_pool(name="ps_l2", bufs=l2bufs, space="PSUM") as ps_l2,
        ):
            w_sb = cpool.tile([128, WCOLS], f32r, tag="wts")
            b_sb = cpool.tile([128, 16], f32, tag="bias")
            x4 = xpool.tile([128, qcols], f32r, tag="x4")
            stage = opool.tile([2 * bankg, 512 * nbank], f32, tag="ostage")

            # DMA order tuned for start latency: the first column blocks
            # (all four quadrants) and the even experts' weights come first;
            # everything else streams behind compute. DRAM tensors are f32r
            # (same bits as f32) so loads ride the sync/HWDGE path.
            kw = bankg * bw               # w2 table cols per expert
            def w1dma(k):
                nc.sync.dma_start(
                    w_sb[:, W1OFF + 128 * k : W1OFF + 128 * (k + 1)],
                    wts[:, W1OFF + 128 * k : W1OFF + 128 * (k + 1)],
                )
            def w2dma(k):
                nc.sync.dma_start(
                    w_sb[:, W2OFF + kw * k : W2OFF + kw * (k + 1)],
                    wts[:, W2OFF + kw * k : W2OFF + kw * (k + 1)],
                )
            def x4dma(c0, c1):
                nc.sync.dma_start(x4[:, c0:c1], x4_c[:, c0:c1])
            nc.sync.dma_start(w_sb[:, W0OFF : W0OFF + 256], wts[:, W0OFF : W0OFF + 256])
            nc.sync.dma_start(x4[0:32, 0:1024], x4_c[0:32, 0:1024])
            nc.sync.dma_start(b_sb[:], bia[:])
            nc.sync.dma_start(x4[32:128, 0:1024], x4_c[32:128, 0:1024])
            w1dma(0)
            w1dma(2); w2dma(0); w2dma(2)
            x4dma(1024, 2048)
            w1dma(4); w2dma(4)
            w1dma(6); w2dma(6)
            for c0 in range(2048, qcols, 2048):
                x4dma(c0, min(c0 + 2048, qcols))
                if c0 == 2048:
                    for k in (1, 3, 5, 7):
                        w1dma(k)
                elif c0 == 4096:
                    for k in (1, 3, 5, 7):
                        w2dma(k)

            # PE warm-up: the cost model runs matmuls at 1.2 GHz until the
            # PE has been continuously busy ~3us. Burn that ramp on zero
            # matmuls against a memset tile while the first DMAs land.
            wz = wpool.tile([128, 256], f32, tag="h0", name="warmzero")
            nc.vector.memset(wz[:], 0.0)
            warm_ps = ps_l2.tile([128, 512], f32, tag="l2", name="warm_ps")
            for wi in range(4):
                nc.tensor.matmul(
                    warm_ps[:, 0:256], wz[:, 0:128], wz[:],
                    start=True, stop=True, tile_position=(0, 0),
                )

            # Software-pipelined emission: engines execute strictly in program
            # order, so at step t we issue h0mm(t), h1mm(t-1), l2mm(t-2) — each
            # instruction's cross-engine dependency then has a full group
            # period to land, and the PE never head-blocks on an evac.
            def kq(g):
                # pair-rotating order: pair p = g//2 takes quadrant p%4 and
                # column blocks 2*(p//4), 2*(p//4)+1 — so the two groups of a
                # pair share the expert (except the 4 pairs straddling the
                # e-boundary at cb=gpe when gpe is odd)
                p = g // 2
                q = p % 4
                cb = 2 * (p // 4) + (g & 1)
                k = 2 * q + (1 if cb >= gpe else 0)
                return k, q, k % 2, 512 * cb

            h0ps_t, h0_t, h1ps_t, h1_t = {}, {}, {}, {}
            l2_ps = [None] * 8
            pending_stage = []
            stage_due = {}

            def _mk(fn, lbl):
                def g(*a, **kw):
                    r = fn(*a, **kw)
                    try:
                        LABELS[r.ins.name] = lbl
                    except Exception:
                        pass
                    return r
                return g

            def _rec(lbl):
                return _mk(nc.tensor.matmul, lbl)

            def _reca(lbl):
                return _mk(nc.scalar.activation, lbl)

            def _recv(lbl):
                return _mk(nc.vector.tensor_scalar, lbl)

            def _recc(lbl):
                return _mk(nc.scalar.copy, lbl)

            def _reccv(lbl):
                return _mk(nc.vector.tensor_copy, lbl)

            def emit_h0mm(g):
                k, q, e, c0 = kq(g)
                h0ps_t[g] = ps_h0.tile([128, 512], f32, tag="h0ps", name=f"h0ps{g}")
                _rec(f"h0mm({g})")(
                    h0ps_t[g][:],
                    w_sb[32 * q : 32 * q + 32, W0OFF + 128 * e : W0OFF + 128 * (e + 1)],
                    x4[32 * q : 32 * q + 32, c0 : c0 + 512],
                    start=True, stop=True, tile_position=(32 * q, 0),
                )

            def emit_h0evac(g):
                k, q, e, c0 = kq(g)
                h0_t[g] = wpool.tile([128, 512], f32r, tag="h0", name=f"h0_{g}")
                _reca(f"h0evac({g})")(
                    h0_t[g][:], h0ps_t.pop(g)[:], Relu, bias=b_sb[:, k : k + 1]
                )

            def emit_h1mm(g):
                k, q, e, c0 = kq(g)
                if not h1pair:
                    h1ps_t[g] = ps_h1.tile(
                        [128, 512], f32, tag="h1ps", name=f"h1ps{g}"
                    )
                    ap = h1ps_t[g][:]
                else:
                    p, half = g // 2, g % 2
                    if half == 0:
                        h1ps_t[p] = ps_h1.tile(
                            [128, 1024], f32, tag="h1ps", name=f"h1ps{p}"
                        )
                    ap = h1ps_t[p][:, 512 * half : 512 * half + 512]
                _rec(f"h1mm({g})")(
                    ap,
                    w_sb[:, W1OFF + 128 * k : W1OFF + 128 * (k + 1)],
                    h0_t.pop(g)[:],
                    start=True, stop=True, tile_position=(0, 0),
                )

            def emit_h1evac(g):
                k, q, e, c0 = kq(g)
                if not h1pair:
                    h1_t[g] = wpool.tile([128, 512], f32r, tag="h1", name=f"h1_{g}")
                    _recv(f"h1evac({g})")(
                        h1_t[g][:], h1ps_t.pop(g)[:], b_sb[:, 8 + k : 9 + k],
                        0.0, add, mx,
                    )
                    return
                if g % 2 == 0:
                    return
                p = g // 2
                ka, kb = kq(g - 1)[0], k
                ps = h1ps_t.pop(p)
                h1_t[p] = wpool.tile([128, 1024], f32r, tag="h1", name=f"h1_{p}")
                if ka == kb:
                    _recv(f"h1evac({g-1},{g})")(
                        h1_t[p][:], ps[:], b_sb[:, 8 + ka : 9 + ka], 0.0, add, mx
                    )
                else:
                    _recv(f"h1evacA({g-1})")(
                        h1_t[p][:, 0:512], ps[:, 0:512],
                        b_sb[:, 8 + ka : 9 + ka], 0.0, add, mx,
                    )
                    _recv(f"h1evacB({g})")(
                        h1_t[p][:, 512:1024], ps[:, 512:1024],
                        b_sb[:, 8 + kb : 9 + kb], 0.0, add, mx,
                    )

            def emit_l2mm(g):
                k, q, e, c0 = kq(g)
                # chain c = g % chains; slot s = (g // chains) % bankg;
                # bank index b = g's window: groups fill `chains` banks per
                # chains*bankg window, then all are staged.
                cch = g % chains
                s = (g // chains) % bankg
                if s == 0 and cch == 0:
                    for i in range(chains):
                        l2_ps[i] = ps_l2.tile(
                            [2 * bankg, 512], f32, tag="l2", name=f"l2_{g}_{i}"
                        )
                tcol = W2OFF + bw * (bankg * k + s)
                last = s == bankg - 1 or g >= ng - chains
                if not h1pair:
                    rhs_ap = h1_t.pop(g)[:]
                else:
                    p, half = g // 2, g % 2
                    rhs_ap = h1_t[p][:, 512 * half : 512 * half + 512]
                    if half == 1:
                        h1_t.pop(p)
                _rec(f"l2mm({g})")(
                    l2_ps[cch][:],
                    w_sb[:, tcol : tcol + bw],
                    rhs_ap,
                    start=(s == 0), stop=last, tile_position=(0, 0),
                )
                if last:
                    b = (g // (chains * bankg)) * chains + cch
                    pending_stage.append((b, l2_ps[cch]))

            def emit_stage():
                b, tile_ = pending_stage.pop(0)
                if stage_eng == "alt":
                    cp = _recc if b % 2 == 0 else _reccv
                else:
                    cp = _recc if stage_eng == "act" else _reccv
                cp(f"stage({b})")(stage[:, 512 * b : 512 * (b + 1)], tile_[:])
                if b == nbank - 2:
                    # second-to-last piece via sync/HWDGE (SP idle by now)
                    o0 = 512 * ((b // 3) * 3)
                    nc.sync.dma_start(out_c[:, o0 : 512 * (b + 1)], stage[:, o0 : 512 * (b + 1)])
                elif b == nbank - 1:
                    # final bank alone: shortest possible critical tail
                    nc.sync.dma_start(out_c[:, 512 * b :], stage[:, 512 * b :])
                elif (b + 1) % 3 == 0:
                    o0 = 512 * ((b // 3) * 3)
                    o1 = 512 * (b + 1)
                    nc.gpsimd.dma_start(out_c[:, o0:o1], stage[:, o0:o1])

            for t in range(ng + l2off + stage_delay + 1):
                if t < ng:
                    emit_h0mm(t)
                if 1 <= t < ng + 1:
                    emit_h1mm(t - 1)
                if l2off <= t < ng + l2off:
                    g2 = t - l2off
                    emit_l2mm(g2)
                    if pending_stage and g2 not in stage_due:
                        # bank finished at step t; stage it stage_delay later
                        stage_due[t + stage_delay] = True
                while pending_stage and (stage_due.pop(t, False) or t >= ng + l2off):
                    emit_stage()
                    break
                if t < ng:
                    emit_h0evac(t)
                if 1 <= t < ng + 1:
                    emit_h1evac(t - 1)

    _split_ctrl_waits(nc, mybir)
    return nc


def _split_ctrl_waits(nc, mybir):
    """walrus in this container accepts only one sync-wait per instruction;
    Tile attaches one wait per dependency lane. Hoist extras onto preceding
    single-wait nops on the same engine (equivalent ordering semantics)."""
    for bb in nc.main_func.blocks:
        newlist = []
        changed = False
        for ins in bb.instructions:
            si = ins.sync_info
            if si is not None and len(si.on_wait) > 1:
                waits = list(si.on_wait)
                for j, w in enumerate(waits[:-1]):
                    nop = mybir.InstNoOp(name=f"{ins.name}-wsplit-{j}", ins=[], outs=[])
                    nop.engine = ins.engine
                    nop.sync_info = mybir.SyncInfo(on_wait=[w], on_update=[])
                    newlist.append(nop)
                si.on_wait = [waits[-1]]
                ins.sync_info = si
                changed = True
            newlist.append(ins)
        if changed:
            bb.instructions = newlist
    return nc


def _prep_consts(W0, b0, W1, b1, W2, b2, bankg=None):
    f = np.float32
    bankg = bankg or BANKG
    bw = 2 * bankg
    wts = np.zeros((128, 1280 + K * bankg * bw), f)
    bia = np.zeros((128, 16), f)
    for k in range(K):
        q, e = k // 2, k % 2
        wts[32 * q : 32 * q + 16, 128 * e : 128 * e + 64] = W0[k]
        wts[32 * q + 16 : 32 * q + 32, 128 * e + 64 : 128 * e + 128] = W0[k]
        wts[0:64, 256 + 128 * k : 256 + 128 * k + 64] = W1[k]
        wts[64:128, 256 + 128 * k + 64 : 256 + 128 * k + 128] = W1[k]
        for s in range(bankg):
            tcol = 1280 + bw * (bankg * k + s)
            wts[0:64, tcol + 2 * s] = W2[k, :, 0]
            wts[64:128, tcol + 2 * s + 1] = W2[k, :, 0]
        bia[0:64, k] = b0[k]
        bia[64:128, k] = b0[k]
        bia[0:64, 8 + k] = b1[k]
        bia[64:128, 8 + k] = b1[k]
    return wts, bia


def kernel(idxs, xs, W0, b0, W1, b1, W2, b2):
    from concourse.bass_utils import run_bass_kernel_spmd

    idx_flat = np.asarray(idxs).reshape(N)
    xs_flat = np.ascontiguousarray(np.asarray(xs, np.float32).reshape(N, D_IN))

    order = np.argsort(idx_flat, kind="stable")
    counts = np.bincount(idx_flat, minlength=K)

    # capacity (multiple of 512 column-pairs) that fits every (core, expert)
    max_part = -(-int(counts.max()) // NCORES)        # samples per (core, expert)
    need = -(-max_part // 2)                          # column-pairs
    cap = max(CAP0, -(-need // 512) * 512)
    if ("nc", cap) not in _cache:
        _cache[("nc", cap)] = _build_nc(cap)
    nc = _cache[("nc", cap)]
    _cache["nc"] = nc                                 # test.py reads this
    qcols = 2 * cap
    gpe = cap // 512
    ng = K * gpe
    nbank = -(-ng // BANKG)

    wts, bia = _prep_consts(
        np.asarray(W0), np.asarray(b0), np.asarray(W1), np.asarray(b1),
        np.asarray(W2), np.asarray(b2),
    )

    # slot[m, k, c, lane] = global sample index routed there (-1 = padding)
    slot = np.full((NCORES, K, cap, 2), -1, np.int64)
    pos = 0
    for k in range(K):
        ids_k = order[pos : pos + counts[k]]
        pos += counts[k]
        for m, p in enumerate(np.array_split(ids_k, NCORES)):
            top, bot = p[0::2], p[1::2]
            slot[m, k, : len(top), 0] = top
            slot[m, k, : len(bot), 1] = bot
    msk = slot >= 0
    xp = np.zeros((NCORES, K, cap, 2, D_IN), np.float32)
    xp[msk] = xs_flat[slot[msk]]

    in_maps = []
    for m in range(NCORES):
        # expert k -> rows 32*(k//2):+32, cols (k%2)*cap
        x4 = xp[m].reshape(4, qcols, 32).transpose(0, 2, 1).reshape(128, qcols)
        in_maps.append(dict(x4_c=np.ascontiguousarray(x4), wts=wts, bia=bia))

    res = run_bass_kernel_spmd(nc, in_maps, list(range(NCORES))).results

    b2v = np.asarray(b2, np.float32)[:, 0]
    # group order is pair-rotating: pair p = g//2 covers quadrant p%4 and
    # column blocks 2*(p//4) + (g%2) (expert k = 2q + (cb>=gpe)); the lane-r
    # value for column c sits at oc[2*(g%BANKG)+r, 512*(g//BANKG)+c]
    out = np.empty(N, np.float32)
    gidx = np.arange(ng)
    p_g = gidx // 2
    q_g = p_g % 4
    cb_g = 2 * (p_g // 4) + (gidx % 2)
    k_g = 2 * q_g + (cb_g >= gpe)
    j_g = cb_g - (k_g % 2) * gpe
    s_g, b_g = gidx % BANKG, gidx // BANKG
    for m in range(NCORES):
        oc = np.asarray(res[m]["out_c"])              # [2*BANKG, 512*nbank]
        vals = np.empty((K, cap, 2), np.float32)
        for g in range(ng):
            blk = oc[2 * s_g[g] : 2 * s_g[g] + 2, 512 * b_g[g] : 512 * b_g[g] + 512]
            vals[k_g[g], 512 * j_g[g] : 512 * (j_g[g] + 1)] = blk.T
        vals = vals + b2v[:, None, None]
        out[slot[m][msk[m]]] = vals[msk[m]]
    return out.reshape(R, S, 1)
